# revision 12
# baseline (speedup 1.0000x reference)
"""CTLSTMCell fused kernel for Trainium2, 8 NeuronCores.

Sharding: tensor-parallel over the D=1024 feature columns. Core c owns
columns [c*128, (c+1)*128) and computes all 7 gate blocks for that slice:
    gates[:, g*1024 + c*128 : g*1024 + (c+1)*128]  for g in 0..6
Each core runs the full batch (B=4096), so the only replicated traffic is
the concatenated input x = [emb, h] (33.5 MB/core); the weight is split
8 ways (7.3 MB/core) and stays resident in SBUF.

On-chip layout is [features, batch] (transposed): the contraction dim K of
the matmul must sit on SBUF partitions for both operands, W is naturally
K-major, and x is transposed once on the host. This also puts the bias on
partitions, so it fuses into the ScalarE activation op (func(scale*in+bias))
for free. Outputs come back [128, 4096] per core and are untransposed on
the host. Matmuls use the float32r dtype (cayman fast-FP32 path: full PE
rate at moving-dim >= 256).
"""

import numpy as np

D = 1024
B = 4096
K = 2 * D            # 2048 contraction
NCORES = 8
DLOC = D // NCORES   # 128 columns of D per core
GCOLS = 7 * DLOC     # 896 gate columns per core
KCH = K // 128       # 16 k-chunks
NT = B // 512        # 8 batch tiles of 512
SCALE = 0.1          # softplus beta

_BUILT = {}


def _build():
    import concourse.bacc as bacc
    import concourse.mybir as mybir
    from concourse.tile import TileContext

    f32r = mybir.dt.float32r
    f32 = mybir.dt.float32
    AF = mybir.ActivationFunctionType

    nc = bacc.Bacc("TRN2")
    xT = nc.declare_dram_parameter("xT", [K, B], f32r, isOutput=False)
    Wc = nc.declare_dram_parameter("Wc", [K, GCOLS], f32r, isOutput=False)
    bc = nc.declare_dram_parameter("bc", [DLOC, 7], f32, isOutput=False)
    cellT = nc.declare_dram_parameter("cellT", [DLOC, B], f32, isOutput=False)
    cellbarT = nc.declare_dram_parameter("cellbarT", [DLOC, B], f32, isOutput=False)
    coT = nc.declare_dram_parameter("coT", [DLOC, B], f32, isOutput=True)
    cboT = nc.declare_dram_parameter("cboT", [DLOC, B], f32, isOutput=True)
    dgoT = nc.declare_dram_parameter("dgoT", [DLOC, B], f32, isOutput=True)
    ogoT = nc.declare_dram_parameter("ogoT", [DLOC, B], f32, isOutput=True)

    # Gate order: dg first (its exp/ln ACTs use the other table set, so
    # leading with it costs one set switch per n-tile), og last (its sigmoid
    # goes straight to DRAM, shortening the kernel tail).
    GORDER = [6, 3, 0, 1, 4, 5, 2]

    with TileContext(nc) as tc:
        with (
            tc.tile_pool(name="wpool", bufs=1) as wp,
            tc.tile_pool(name="xpool", bufs=2) as xp,
            tc.tile_pool(name="gpool", bufs=2) as gp,
            tc.tile_pool(name="tpool", bufs=1) as tp,
            tc.tile_pool(name="opool", bufs=2) as op_,
            tc.tile_pool(name="pspool", bufs=8, space="PSUM") as pp,
        ):
            # W chunks and the first x tile, interleaved per k-chunk so the
            # first matmuls start as soon as chunk 0 of each has landed
            # (separate tiles per chunk -> per-chunk DMA deps).
            def load_x_chunks(n):
                ns = slice(n * 512, (n + 1) * 512)
                xts = []
                for kc in range(KCH):
                    xk = xp.tile([128, 512], f32r, tag=f"x{kc}", name=f"x_{n}_{kc}")
                    nc.sync.dma_start(out=xk[:, :], in_=xT[kc * 128:(kc + 1) * 128, ns])
                    xts.append(xk)
                return xts

            wts = []
            xnext = []
            for kc in range(KCH):
                wk = wp.tile([128, GCOLS], f32r, tag=f"w{kc}", name=f"w_{kc}")
                nc.sync.dma_start(out=wk[:, :], in_=Wc[kc * 128:(kc + 1) * 128, :])
                wts.append(wk)
                xk = xp.tile([128, 512], f32r, tag=f"x{kc}", name=f"x_0_{kc}")
                # scalar is the other HW-DGE trigger engine and is idle until
                # the first activation (~42us): issuing x0 here runs the
                # trigger stream at 2x rate, so the HBM ramp saturates sooner
                nc.scalar.dma_start(out=xk[:, :], in_=xT[kc * 128:(kc + 1) * 128, 0:512])
                xnext.append(xk)

            bt = wp.tile([128, 7], f32)
            nc.sync.dma_start(out=bt[:, :], in_=bc[:, :])

            for n in range(NT):
                ns = slice(n * 512, (n + 1) * 512)
                xts = xnext

                if n + 1 < NT:
                    xnext = load_x_chunks(n + 1)

                ct = gp.tile([128, 512], f32, tag="ct")
                nc.scalar.dma_start(out=ct[:, :], in_=cellT[:, ns])
                cbt = gp.tile([128, 512], f32, tag="cbt")
                nc.scalar.dma_start(out=cbt[:, :], in_=cellbarT[:, ns])

                # k-chunk outer, gate inner: all 7 PSUM banks accumulate in
                # lockstep, so the stream is paced by the chunk DMAs instead
                # of serializing a whole gate behind them. The last n-tile
                # runs gate-outer instead: each gate finishes as early as
                # possible so only og's ACT+store trail the final matmul.
                pts = {
                    g: pp.tile([128, 512], f32, tag="pt", name=f"pt_{n}_{g}")
                    for g in GORDER
                }
                if n < NT - 1:
                    loop = [(kc, g) for kc in range(KCH) for g in GORDER]
                else:
                    loop = [(kc, g) for g in GORDER for kc in range(KCH)]
                for kc, g in loop:
                    nc.tensor.matmul(
                        pts[g][:, :],
                        wts[kc][:, g * 128:(g + 1) * 128],
                        xts[kc][:, :],
                        start=(kc == 0),
                        stop=(kc == KCH - 1),
                    )

                # softplus(SCALE*d) = ln(1 + exp(SCALE*d)) — the toolchain's
                # ACT tables have no softplus entry, but exp and ln share a
                # table set. bc[:, 6] is pre-scaled by SCALE on the host; the
                # /SCALE lands on the DVE below.
                ept = tp.tile([128, 512], f32, tag="ept")
                nc.scalar.activation(
                    ept[:, :], pts[6][:, :], AF.Exp, bias=bt[:, 6:7], scale=SCALE
                )
                spt = gp.tile([128, 512], f32, tag="spt")
                nc.scalar.activation(spt[:, :], ept[:, :], AF.Ln, bias=1.0)
                dgt = op_.tile([128, 512], f32, tag="dgt")
                nc.vector.tensor_scalar_mul(dgt[:, :], spt[:, :], 1.0 / SCALE)
                nc.sync.dma_start(out=dgoT[:, ns], in_=dgt[:, :])

                cin = gp.tile([128, 512], f32, tag="cin")
                nc.scalar.activation(cin[:, :], pts[3][:, :], AF.Tanh, bias=bt[:, 3:4])
                s_ig = gp.tile([128, 512], f32, tag="s_ig")
                nc.scalar.activation(s_ig[:, :], pts[0][:, :], AF.Sigmoid, bias=bt[:, 0:1])
                s_fg = gp.tile([128, 512], f32, tag="s_fg")
                nc.scalar.activation(s_fg[:, :], pts[1][:, :], AF.Sigmoid, bias=bt[:, 1:2])

                t1 = tp.tile([128, 512], f32, tag="t1")
                nc.vector.tensor_mul(t1[:, :], s_fg[:, :], ct[:, :])
                t2 = tp.tile([128, 512], f32, tag="t2")
                nc.vector.tensor_mul(t2[:, :], s_ig[:, :], cin[:, :])
                cot = op_.tile([128, 512], f32, tag="cot")
                nc.vector.tensor_add(cot[:, :], t1[:, :], t2[:, :])
                nc.sync.dma_start(out=coT[:, ns], in_=cot[:, :])

                s_ibg = gp.tile([128, 512], f32, tag="s_ibg")
                nc.scalar.activation(s_ibg[:, :], pts[4][:, :], AF.Sigmoid, bias=bt[:, 4:5])
                s_fbg = gp.tile([128, 512], f32, tag="s_fbg")
                nc.scalar.activation(s_fbg[:, :], pts[5][:, :], AF.Sigmoid, bias=bt[:, 5:6])

                t3 = tp.tile([128, 512], f32, tag="t3")
                nc.vector.tensor_mul(t3[:, :], s_fbg[:, :], cbt[:, :])
                t4 = tp.tile([128, 512], f32, tag="t4")
                nc.vector.tensor_mul(t4[:, :], s_ibg[:, :], cin[:, :])
                cbot = op_.tile([128, 512], f32, tag="cbot")
                nc.vector.tensor_add(cbot[:, :], t3[:, :], t4[:, :])
                nc.sync.dma_start(out=cboT[:, ns], in_=cbot[:, :])

                ogt = op_.tile([128, 512], f32, tag="ogt")
                nc.scalar.activation(ogt[:, :], pts[2][:, :], AF.Sigmoid, bias=bt[:, 2:3])
                nc.sync.dma_start(out=ogoT[:, ns], in_=ogt[:, :])

    nc.compile()
    return nc


def get_nc():
    if "nc" not in _BUILT:
        _BUILT["nc"] = _build()
    return _BUILT["nc"]


def make_in_maps(event_type_emb_i, hidden_t__i_minus_1, cell_t__i_minus_1,
                 cell_bar_i_minus_1, W, b):
    emb = np.asarray(event_type_emb_i, dtype=np.float32)
    h = np.asarray(hidden_t__i_minus_1, dtype=np.float32)
    cell = np.asarray(cell_t__i_minus_1, dtype=np.float32)
    cellbar = np.asarray(cell_bar_i_minus_1, dtype=np.float32)
    W = np.asarray(W, dtype=np.float32)
    b = np.asarray(b, dtype=np.float32)

    xT = np.ascontiguousarray(np.concatenate([emb, h], axis=1).T)  # [2048, 4096]
    cellT = np.ascontiguousarray(cell.T)        # [1024, 4096]
    cellbarT = np.ascontiguousarray(cellbar.T)  # [1024, 4096]

    in_maps = []
    for c in range(NCORES):
        cols = np.concatenate(
            [np.arange(g * D + c * DLOC, g * D + (c + 1) * DLOC) for g in range(7)]
        )
        Wc = np.ascontiguousarray(W[:, cols])            # [2048, 896]
        bc = np.ascontiguousarray(b[cols].reshape(7, DLOC).T)  # [128, 7]
        bc[:, 6] *= SCALE
        in_maps.append({
            "xT": xT,
            "Wc": Wc,
            "bc": bc,
            "cellT": np.ascontiguousarray(cellT[c * DLOC:(c + 1) * DLOC, :]),
            "cellbarT": np.ascontiguousarray(cellbarT[c * DLOC:(c + 1) * DLOC, :]),
        })
    return in_maps


def assemble(results):
    outs = []
    for name in ("coT", "cboT", "dgoT", "ogoT"):
        full = np.empty((B, D), dtype=np.float32)
        for c, r in enumerate(results):
            full[:, c * DLOC:(c + 1) * DLOC] = r[name].T
        outs.append(full)
    return tuple(outs)


def kernel(**inputs):
    from concourse.bass_utils import run_bass_kernel_spmd

    nc = get_nc()
    in_maps = make_in_maps(**inputs)
    res = run_bass_kernel_spmd(nc, in_maps, list(range(NCORES)))
    return assemble(res.results)


# revision 13
# speedup vs baseline: 1.0244x; 1.0244x over previous
"""CTLSTMCell fused kernel for Trainium2, 8 NeuronCores.

Sharding: tensor-parallel over the D=1024 feature columns. Core c owns
columns [c*128, (c+1)*128) and computes all 7 gate blocks for that slice:
    gates[:, g*1024 + c*128 : g*1024 + (c+1)*128]  for g in 0..6
Each core runs the full batch (B=4096), so the only replicated traffic is
the concatenated input x = [emb, h] (33.5 MB/core); the weight is split
8 ways (7.3 MB/core) and stays resident in SBUF.

On-chip layout is [features, batch] (transposed): the contraction dim K of
the matmul must sit on SBUF partitions for both operands, W is naturally
K-major, and x is transposed once on the host. This also puts the bias on
partitions, so it fuses into the ScalarE activation op (func(scale*in+bias))
for free. Outputs come back [128, 4096] per core and are untransposed on
the host. Matmuls use the float32r dtype (cayman fast-FP32 path: full PE
rate at moving-dim >= 256).
"""

import numpy as np

D = 1024
B = 4096
K = 2 * D            # 2048 contraction
NCORES = 8
DLOC = D // NCORES   # 128 columns of D per core
GCOLS = 7 * DLOC     # 896 gate columns per core
KCH = K // 128       # 16 k-chunks
NT = B // 512        # 8 batch tiles of 512
SCALE = 0.1          # softplus beta

_BUILT = {}


def _build():
    import concourse.bacc as bacc
    import concourse.mybir as mybir
    from concourse.tile import TileContext

    f32r = mybir.dt.float32r
    f32 = mybir.dt.float32
    AF = mybir.ActivationFunctionType

    nc = bacc.Bacc("TRN2")
    xT = nc.declare_dram_parameter("xT", [K, B], f32r, isOutput=False)
    Wc = nc.declare_dram_parameter("Wc", [K, GCOLS], f32r, isOutput=False)
    bc = nc.declare_dram_parameter("bc", [DLOC, 7], f32, isOutput=False)
    cellT = nc.declare_dram_parameter("cellT", [DLOC, B], f32, isOutput=False)
    cellbarT = nc.declare_dram_parameter("cellbarT", [DLOC, B], f32, isOutput=False)
    coT = nc.declare_dram_parameter("coT", [DLOC, B], f32, isOutput=True)
    cboT = nc.declare_dram_parameter("cboT", [DLOC, B], f32, isOutput=True)
    dgoT = nc.declare_dram_parameter("dgoT", [DLOC, B], f32, isOutput=True)
    ogoT = nc.declare_dram_parameter("ogoT", [DLOC, B], f32, isOutput=True)

    # Gate order: dg first (its exp/ln ACTs use the other table set, so
    # leading with it costs one set switch per n-tile), og last (its sigmoid
    # goes straight to DRAM, shortening the kernel tail).
    GORDER = [6, 3, 0, 1, 4, 5, 2]

    with TileContext(nc) as tc:
        with (
            tc.tile_pool(name="wpool", bufs=1) as wp,
            tc.tile_pool(name="xpool", bufs=2) as xp,
            tc.tile_pool(name="gpool", bufs=2) as gp,
            tc.tile_pool(name="tpool", bufs=1) as tp,
            tc.tile_pool(name="opool", bufs=2) as op_,
            tc.tile_pool(name="pspool", bufs=8, space="PSUM") as pp,
        ):
            # W chunks and the first x tile, interleaved per k-chunk so the
            # first matmuls start as soon as chunk 0 of each has landed
            # (separate tiles per chunk -> per-chunk DMA deps).
            def load_x_chunks(n):
                ns = slice(n * 512, (n + 1) * 512)
                xts = []
                for kc in range(KCH):
                    xk = xp.tile([128, 512], f32r, tag=f"x{kc}", name=f"x_{n}_{kc}")
                    nc.sync.dma_start(out=xk[:, :], in_=xT[kc * 128:(kc + 1) * 128, ns])
                    xts.append(xk)
                return xts

            wts = []
            xnext = []
            for kc in range(KCH):
                wk = wp.tile([128, GCOLS], f32r, tag=f"w{kc}", name=f"w_{kc}")
                nc.sync.dma_start(out=wk[:, :], in_=Wc[kc * 128:(kc + 1) * 128, :])
                wts.append(wk)
                xk = xp.tile([128, 512], f32r, tag=f"x{kc}", name=f"x_0_{kc}")
                nc.sync.dma_start(out=xk[:, :], in_=xT[kc * 128:(kc + 1) * 128, 0:512])
                xnext.append(xk)

            bt = wp.tile([128, 7], f32)
            nc.sync.dma_start(out=bt[:, :], in_=bc[:, :])

            for n in range(NT):
                ns = slice(n * 512, (n + 1) * 512)
                xts = xnext

                if n + 1 < NT:
                    xnext = load_x_chunks(n + 1)

                ct = gp.tile([128, 512], f32, tag="ct")
                nc.sync.dma_start(out=ct[:, :], in_=cellT[:, ns])
                cbt = gp.tile([128, 512], f32, tag="cbt")
                nc.sync.dma_start(out=cbt[:, :], in_=cellbarT[:, ns])

                # k-chunk outer, gate inner: all 7 PSUM banks accumulate in
                # lockstep, so the stream is paced by the chunk DMAs instead
                # of serializing a whole gate behind them. The last n-tile
                # runs gate-outer instead: each gate finishes as early as
                # possible so only og's ACT+store trail the final matmul.
                pts = {
                    g: pp.tile([128, 512], f32, tag="pt", name=f"pt_{n}_{g}")
                    for g in GORDER
                }
                if n < NT - 1:
                    loop = [(kc, g) for kc in range(KCH) for g in GORDER]
                else:
                    loop = [(kc, g) for g in GORDER for kc in range(KCH)]
                for kc, g in loop:
                    nc.tensor.matmul(
                        pts[g][:, :],
                        wts[kc][:, g * 128:(g + 1) * 128],
                        xts[kc][:, :],
                        start=(kc == 0),
                        stop=(kc == KCH - 1),
                    )

                # softplus(SCALE*d) = ln(1 + exp(SCALE*d)) — the toolchain's
                # ACT tables have no softplus entry, but exp and ln share a
                # table set. bc[:, 6] is pre-scaled by SCALE on the host; the
                # /SCALE lands on the DVE below.
                ept = tp.tile([128, 512], f32, tag="ept")
                nc.scalar.activation(
                    ept[:, :], pts[6][:, :], AF.Exp, bias=bt[:, 6:7], scale=SCALE
                )
                spt = gp.tile([128, 512], f32, tag="spt")
                nc.scalar.activation(spt[:, :], ept[:, :], AF.Ln, bias=1.0)
                dgt = op_.tile([128, 512], f32, tag="dgt")
                nc.vector.tensor_scalar_mul(dgt[:, :], spt[:, :], 1.0 / SCALE)
                nc.sync.dma_start(out=dgoT[:, ns], in_=dgt[:, :])

                cin = gp.tile([128, 512], f32, tag="cin")
                nc.scalar.activation(cin[:, :], pts[3][:, :], AF.Tanh, bias=bt[:, 3:4])
                s_ig = gp.tile([128, 512], f32, tag="s_ig")
                nc.scalar.activation(s_ig[:, :], pts[0][:, :], AF.Sigmoid, bias=bt[:, 0:1])
                s_fg = gp.tile([128, 512], f32, tag="s_fg")
                nc.scalar.activation(s_fg[:, :], pts[1][:, :], AF.Sigmoid, bias=bt[:, 1:2])

                t1 = tp.tile([128, 512], f32, tag="t1")
                nc.vector.tensor_mul(t1[:, :], s_fg[:, :], ct[:, :])
                t2 = tp.tile([128, 512], f32, tag="t2")
                nc.vector.tensor_mul(t2[:, :], s_ig[:, :], cin[:, :])
                cot = op_.tile([128, 512], f32, tag="cot")
                nc.vector.tensor_add(cot[:, :], t1[:, :], t2[:, :])
                nc.sync.dma_start(out=coT[:, ns], in_=cot[:, :])

                s_ibg = gp.tile([128, 512], f32, tag="s_ibg")
                nc.scalar.activation(s_ibg[:, :], pts[4][:, :], AF.Sigmoid, bias=bt[:, 4:5])
                s_fbg = gp.tile([128, 512], f32, tag="s_fbg")
                nc.scalar.activation(s_fbg[:, :], pts[5][:, :], AF.Sigmoid, bias=bt[:, 5:6])

                t3 = tp.tile([128, 512], f32, tag="t3")
                nc.vector.tensor_mul(t3[:, :], s_fbg[:, :], cbt[:, :])
                t4 = tp.tile([128, 512], f32, tag="t4")
                nc.vector.tensor_mul(t4[:, :], s_ibg[:, :], cin[:, :])
                cbot = op_.tile([128, 512], f32, tag="cbot")
                nc.vector.tensor_add(cbot[:, :], t3[:, :], t4[:, :])
                nc.sync.dma_start(out=cboT[:, ns], in_=cbot[:, :])

                ogt = op_.tile([128, 512], f32, tag="ogt")
                nc.scalar.activation(ogt[:, :], pts[2][:, :], AF.Sigmoid, bias=bt[:, 2:3])
                nc.sync.dma_start(out=ogoT[:, ns], in_=ogt[:, :])

    nc.compile()
    return nc


def get_nc():
    if "nc" not in _BUILT:
        _BUILT["nc"] = _build()
    return _BUILT["nc"]


def make_in_maps(event_type_emb_i, hidden_t__i_minus_1, cell_t__i_minus_1,
                 cell_bar_i_minus_1, W, b):
    emb = np.asarray(event_type_emb_i, dtype=np.float32)
    h = np.asarray(hidden_t__i_minus_1, dtype=np.float32)
    cell = np.asarray(cell_t__i_minus_1, dtype=np.float32)
    cellbar = np.asarray(cell_bar_i_minus_1, dtype=np.float32)
    W = np.asarray(W, dtype=np.float32)
    b = np.asarray(b, dtype=np.float32)

    xT = np.ascontiguousarray(np.concatenate([emb, h], axis=1).T)  # [2048, 4096]
    cellT = np.ascontiguousarray(cell.T)        # [1024, 4096]
    cellbarT = np.ascontiguousarray(cellbar.T)  # [1024, 4096]

    in_maps = []
    for c in range(NCORES):
        cols = np.concatenate(
            [np.arange(g * D + c * DLOC, g * D + (c + 1) * DLOC) for g in range(7)]
        )
        Wc = np.ascontiguousarray(W[:, cols])            # [2048, 896]
        bc = np.ascontiguousarray(b[cols].reshape(7, DLOC).T)  # [128, 7]
        bc[:, 6] *= SCALE
        in_maps.append({
            "xT": xT,
            "Wc": Wc,
            "bc": bc,
            "cellT": np.ascontiguousarray(cellT[c * DLOC:(c + 1) * DLOC, :]),
            "cellbarT": np.ascontiguousarray(cellbarT[c * DLOC:(c + 1) * DLOC, :]),
        })
    return in_maps


def assemble(results):
    outs = []
    for name in ("coT", "cboT", "dgoT", "ogoT"):
        full = np.empty((B, D), dtype=np.float32)
        for c, r in enumerate(results):
            full[:, c * DLOC:(c + 1) * DLOC] = r[name].T
        outs.append(full)
    return tuple(outs)


def kernel(**inputs):
    from concourse.bass_utils import run_bass_kernel_spmd

    nc = get_nc()
    in_maps = make_in_maps(**inputs)
    res = run_bass_kernel_spmd(nc, in_maps, list(range(NCORES)))
    return assemble(res.results)
